# revision 1
# baseline (speedup 1.0000x reference)
"""Trainium2 Bass kernel for nn_AutoregressiveBisectionInverter.

Inverts y = softplus(s)*x + 0.1*x^3 + tanh(W@x + b) (W strictly lower
triangular) per batch row.  Since W is strictly lower-triangular, the tanh
term at position i depends only on already-solved x_{<i}; each position is
a monotone-cubic scalar root solve.

Strategy (per NeuronCore, batch sharded 1024 -> 8 x 128 rows on the 128
SBUF partitions):
  - Normalize:  x = sqrt(abar)*v with abar = 10*softplus(s)  so the cubic
    becomes p(v) = v^3 + v + dt  (unit coefficients, p' >= 1, |root| <= VM).
  - Per autoregressive step i (serial DVE chain + ScalarE leg):
      ScalarE: tanh_i = Tanh(W'[i,i-1]*v_{i-1} + cb)  -- the last dot term
               rides tanh's scale, cb = (partial dot + b_i) comes from a
               Copy+accum_out reduce seeded with bias=b_i/D;
               nd = Yt[:,i] - kappa_i*tanh_i  (Identity activation)
      DVE:  cnt = #{k: u_k < nd} + seed   (ONE tensor_scalar is_lt+accum over
              a host-baked grid u_k = p-poly(v_k); exact fp32 count ~ 7
              bisection steps)
            two Newton polish rounds, each as: Horner scan (den), reciprocal,
            Horner scan (num), multiply -- tensor_tensor_scan with a
            stride-0 free-axis broadcast of v evaluates 3v^2+1 and 2v^3+nd
            in one instruction each; round 1 runs in count units with the
            grid pitch h1 folded into the scan initial values.
      The [128,32] partial-dot multiply for row i+1 runs on DVE during step
      i's tanh window (column i of v is still zero there).
  - Output x = sqrt(abar)*v (one elementwise mult), DMA out.

Raw bass Blocks are used (TileContext's tail drain trips a sync-wait limit
in this walrus build), with explicit drain() between every same-engine
producer->consumer pair (DVE/ACT pipelines do not interlock RAW hazards).
All input-dependent scalars are baked as instruction immediates at trace
time; broadcasts/grids are precomputed on the host and DMA'd in dependency
order so compute starts after the first small loads.
"""

import numpy as np

B, D = 1024, 32
NCORES = 8
ROWS = B // NCORES  # 128 rows per core == SBUF partitions
N1 = 96             # bisection-grid points in the fused count op


def _softplus64(x):
    x = x.astype(np.float64)
    return np.log1p(np.exp(-np.abs(x))) + np.maximum(x, 0)


def build(y, W, s, b):
    """Build the SPMD Bass program; returns (nc, in_maps)."""
    from contextlib import ExitStack
    import concourse.bass as bass
    from concourse import mybir

    f32 = mybir.dt.float32
    Alu = mybir.AluOpType
    Act = mybir.ActivationFunctionType

    y = np.ascontiguousarray(np.asarray(y), dtype=np.float32)
    W64 = np.asarray(W, dtype=np.float64)
    s64 = np.asarray(s, dtype=np.float64)
    b64 = np.asarray(b, dtype=np.float64)

    # ---- host precompute ----
    abar = 10.0 * _softplus64(s64)                 # v-linear coefficient
    sqrt_abar = np.sqrt(abar)
    kappa = (10.0 * abar ** -1.5).astype(np.float32)     # per-step immediates
    Yt = (10.0 * y.astype(np.float64) * abar[None, :] ** -1.5).astype(np.float32)
    Wp = np.ascontiguousarray((W64 * sqrt_abar[None, :]).astype(np.float32))
    SA = sqrt_abar.astype(np.float32)[None, :]            # [1, D]
    BT = b64.astype(np.float32)[None, :]                  # [1, D] tanh bias

    dmax = 10.0 * (1.0 + np.abs(y).max(axis=0)) * abar ** -1.5
    VM = float(np.max(np.minimum(np.cbrt(dmax), dmax)) * 1.02 + 1e-3)
    H1 = float(np.float32(2 * VM / (N1 - 1)))
    VM = float(np.float32(VM))
    vk = (-VM + np.arange(N1, dtype=np.float64) * H1)
    UG = ((vk * vk + 1.0) * vk).astype(np.float32)[None, :]   # [1, N1] p-poly
    SEED = float(np.float32(-VM / H1 - 0.5))  # v0 = (count + SEED) * H1

    # One header array per core: [ ytt | btt | sat | ugt ] columns, plus a
    # pre-broadcast W' -- exactly two input DMAs (DMA cost here is dominated
    # by the 128 per-partition descriptors, not bytes).
    HW = 3 * D + N1
    WPB = np.ascontiguousarray(np.broadcast_to(Wp[None, :, :], (ROWS, D, D)))

    # ---- build the SPMD Bass program (input-dependent immediates baked) ----
    nc = bass.Bass()
    hd_d = nc.dram_tensor("hdr", [ROWS, HW], f32, kind="ExternalInput")
    wp_d = nc.dram_tensor("wpb", [ROWS, D, D], f32, kind="ExternalInput")
    xo_d = nc.dram_tensor("xout", [ROWS, D], f32, kind="ExternalOutput")

    def frep(ap, k):
        # broadcast a [P,1] AP along the free axis via stride 0
        return bass.AP(tensor=ap.tensor, offset=ap.offset,
                       ap=[list(ap.ap[0]), [0, k]])

    with ExitStack() as ctx:
        v = ctx.enter_context(nc.sbuf_tensor([ROWS, D], f32))       # v-space solution
        wp = ctx.enter_context(nc.sbuf_tensor([ROWS, D, D], f32))   # W' bcast
        hdr = ctx.enter_context(nc.sbuf_tensor([ROWS, HW], f32))
        ytt = hdr[:, 0:D]
        btt = hdr[:, D:2 * D]
        sat = hdr[:, 2 * D:3 * D]
        ugt = hdr[:, 3 * D:3 * D + N1]
        xo = ctx.enter_context(nc.sbuf_tensor([ROWS, D], f32))
        gs = ctx.enter_context(nc.sbuf_tensor([ROWS, N1], f32))     # count scratch
        prod = ctx.enter_context(nc.sbuf_tensor([ROWS, D], f32))
        junk = ctx.enter_context(nc.sbuf_tensor([ROWS, D], f32))
        c = ctx.enter_context(nc.sbuf_tensor([ROWS, 1], f32))
        t = ctx.enter_context(nc.sbuf_tensor([ROWS, 1], f32))
        cb = ctx.enter_context(nc.sbuf_tensor([ROWS, 1], f32))      # cpart + b_i
        cnt = ctx.enter_context(nc.sbuf_tensor([ROWS, 1], f32))
        ndt = ctx.enter_context(nc.sbuf_tensor([ROWS, 3], f32))     # [0,0,nd]
        dden = ctx.enter_context(nc.sbuf_tensor([ROWS, 2], f32))    # [0,1]
        scd = ctx.enter_context(nc.sbuf_tensor([ROWS, 2], f32))     # den scan out
        scn = ctx.enter_context(nc.sbuf_tensor([ROWS, 3], f32))     # num scan out
        r = ctx.enter_context(nc.sbuf_tensor([ROWS, 1], f32))
        v1 = ctx.enter_context(nc.sbuf_tensor([ROWS, 1], f32))
        s_dma = ctx.enter_context(nc.semaphore("s_dma"))
        s_dve = ctx.enter_context(nc.semaphore("s_dve"))
        s_act = ctx.enter_context(nc.semaphore("s_act"))
        s_gp = ctx.enter_context(nc.semaphore("s_gp"))
        s_r = ctx.enter_context(nc.semaphore("s_r"))
        s_v = ctx.enter_context(nc.semaphore("s_v"))
        block = ctx.enter_context(nc.Block())

        @block.sync
        def _(sync):
            # final store: wait for the vector chain's last inc
            sync.wait_ge(s_dve, 2)
            sync.dma_start(out=xo_d[:, :], in_=xo[:, :]).then_inc(s_dma, 16)
            sync.wait_ge(s_dma, 48)

        @block.gpsimd
        def _(gpsimd):
            gpsimd.dma_start(out=hdr[:, :], in_=hd_d[:, :]).then_inc(s_dma, 16)
            gpsimd.dma_start(out=wp[:, :, :], in_=wp_d[:, :, :]).then_inc(s_dma, 16)

        # NOTE: DVE/ACT pipelines do not interlock same-engine RAW hazards in
        # raw bass -- a dependent back-to-back op reads stale SBUF.  Every
        # producer->consumer edge needs a drain() (pipeline flush) between.
        @block.vector
        def _(vector):
            nc.vector.memset(v[:, :], 0.0)
            nc.vector.memset(c[:, :], 0.0)
            nc.vector.memset(ndt[:, :], 0.0)
            nc.vector.memset(dden[:, 0:1], 0.0)
            nc.vector.memset(dden[:, 1:2], 1.0)
            nc.vector.drain().then_inc(s_dve, 1)  # c_0 = 0 / const tiles ready
            vector.wait_ge(s_dma, 16)  # header (ytt/btt/sat/ugt) landed
            for i in range(D):
                if 1 <= i <= D - 2:
                    # speculative partial-dot multiply for row i+1; runs under
                    # tanh_i (column i of v is still zero).  The free-axis sum
                    # happens on the otherwise-idle ScalarE.
                    if i == 1:
                        vector.wait_ge(s_dma, 32)  # W' landed
                    if i >= 2:
                        vector.wait_ge(s_r, i - 1)  # ScalarE consumed prod row i
                    nc.vector.tensor_mul(prod[:, :], v[:, :], wp[:, i + 1, :])
                    nc.vector.drain().then_inc(s_gp, 1)
                vector.wait_ge(s_act, i + 1)  # tanh_i + nd affine done
                # count = #{u_k < nd} + SEED  (exact fp32 integer count)
                nc.vector.tensor_scalar(
                    out=gs[:, :], in0=ugt[:, :], scalar1=ndt[:, 2:3],
                    scalar2=SEED, op0=Alu.is_lt, op1=Alu.add,
                    accum_out=cnt[:, :])
                nc.vector.drain()
                # Newton round 1 in count units (v0 = cnt*H1); Horner scans:
                #   den = (3*H1^2*cnt)*cnt + 1 ; num = ((2*H1^3*cnt)*cnt)*cnt + nd
                nc.vector.tensor_tensor_scan(
                    out=scd[:, :], data0=frep(cnt[:, 0:1], 2), data1=dden[:, :],
                    initial=float(3 * H1 * H1), op0=Alu.mult, op1=Alu.add)
                nc.vector.drain()
                nc.vector.reciprocal(out=r[:, :], in_=scd[:, 1:2])
                nc.vector.tensor_tensor_scan(
                    out=scn[:, :], data0=frep(cnt[:, 0:1], 3), data1=ndt[:, :],
                    initial=float(2 * H1 ** 3), op0=Alu.mult, op1=Alu.add)
                nc.vector.drain()
                nc.vector.tensor_mul(v1[:, :], scn[:, 2:3], r[:, :])
                nc.vector.drain()
                # Newton round 2 -> write v[:, i]
                nc.vector.tensor_tensor_scan(
                    out=scd[:, :], data0=frep(v1[:, 0:1], 2), data1=dden[:, :],
                    initial=3.0, op0=Alu.mult, op1=Alu.add)
                nc.vector.drain()
                nc.vector.reciprocal(out=r[:, :], in_=scd[:, 1:2])
                nc.vector.tensor_tensor_scan(
                    out=scn[:, :], data0=frep(v1[:, 0:1], 3), data1=ndt[:, :],
                    initial=2.0, op0=Alu.mult, op1=Alu.add)
                nc.vector.drain()
                nc.vector.tensor_mul(v[:, i:i + 1], scn[:, 2:3], r[:, :])
                if i <= D - 2:
                    nc.vector.drain().then_inc(s_v, 1)
                else:
                    nc.vector.drain()
            nc.vector.tensor_mul(xo[:, :], v[:, :], sat[:, :])
            nc.vector.drain().then_inc(s_dve, 1)

        @block.scalar
        def _(scalar):
            scalar.wait_ge(s_dma, 16)  # header landed
            for i in range(D):
                if i >= 2:
                    # cb = (partial dot of row i) + b_i : Copy+accum with the
                    # per-element bias b_i/D so the sum carries the tanh bias.
                    scalar.wait_ge(s_gp, i - 1)
                    nc.scalar.activation(
                        out=junk[:, :], in_=prod[:, :], func=Act.Copy,
                        bias=float(b64[i] / D), scale=1.0,
                        accum_out=cb[:, :])
                    nc.scalar.drain().then_inc(s_r, 1)
                # tanh_i; the last dot term W'[i,i-1]*v_{i-1} rides the scale
                if i == 0:
                    scalar.wait_ge(s_dve, 1)
                    nc.scalar.activation(
                        out=t[:, :], in_=c[:, :], func=Act.Tanh,
                        bias=btt[:, 0:1], scale=1.0)
                elif i == 1:
                    scalar.wait_ge(s_v, 1)
                    nc.scalar.activation(
                        out=t[:, :], in_=v[:, 0:1], func=Act.Tanh,
                        bias=btt[:, 1:2], scale=float(Wp[1, 0]))
                else:
                    scalar.wait_ge(s_v, i)
                    nc.scalar.activation(
                        out=t[:, :], in_=v[:, i - 1:i], func=Act.Tanh,
                        bias=cb[:, :], scale=float(Wp[i, i - 1]))
                nc.scalar.drain()
                # nd = Yt[:,i] - kappa_i * tanh(...), written into ndt[:,2]
                nc.scalar.activation(
                    out=ndt[:, 2:3], in_=t[:, :], func=Act.Identity,
                    bias=ytt[:, i:i + 1], scale=float(-kappa[i]))
                nc.scalar.drain().then_inc(s_act, 1)

    in_maps = []
    for c0 in range(NCORES):
        hdr_np = np.concatenate([
            Yt[c0 * ROWS:(c0 + 1) * ROWS],
            np.broadcast_to(BT, (ROWS, D)),
            np.broadcast_to(SA, (ROWS, D)),
            np.broadcast_to(UG, (ROWS, N1)),
        ], axis=1)
        in_maps.append({"hdr": np.ascontiguousarray(hdr_np), "wpb": WPB})
    return nc, in_maps


def kernel(y, W, s, b):
    from concourse.bass_utils import run_bass_kernel_spmd

    nc, in_maps = build(y, W, s, b)
    res = run_bass_kernel_spmd(nc, in_maps, list(range(NCORES))).results
    X = np.concatenate([res[c]["xout"] for c in range(NCORES)], axis=0)
    return X.astype(np.float32)


if __name__ == "__main__":
    rng = np.random.default_rng(0)
    y = rng.standard_normal((B, D)).astype(np.float32)
    W = np.tril(rng.standard_normal((D, D)), -1).astype(np.float32) * 0.5
    s = rng.standard_normal(D).astype(np.float32)
    b = rng.standard_normal(D).astype(np.float32)
    X = kernel(y=y, W=W, s=s, b=b)
    print("out", X.shape, X.dtype, X[0, :4])



# revision 2
# speedup vs baseline: 1.1565x; 1.1565x over previous
"""Trainium2 Bass kernel for nn_AutoregressiveBisectionInverter (v5).

Closed-form cubic root per autoregressive step: with x the solution of
softplus(s)x + 0.1x^3 + tanh(Wx+b) = y (W strictly lower triangular), each
step solves v^3 + v = nd exactly via v = (t^(1/3)-t^(-1/3))/sqrt(3),
t = z + sqrt(z^2+1), z = (3*sqrt(3)/2)*nd, and x_k = sat_k*v (sat folded
into the Exp biases so SBUF holds x directly).

v5 removes ALL drains: every RAW edge is a semaphore edge (then_inc on the
producer, _wait_ge on a consumer). Hardware allows ONE wait per
instruction, so the schedule uses transitivity on the in-order engines:
an op whose producer is already gated by an earlier op's wait needs no
wait of its own. The per-step ACT chain is 8 back-to-back ops (456ns of
sequencer issue, engines ~idle):

   tanh[w sA=sub_{k-1}] -> z=Identity[w sA=tanh] -> Square(no wait)
   -> Sqrt[w sA=sq] -> Ln(bias=z)[w sA=sqrt] -> Exp[w sA=ln]
   -> Exp(no sA wait; carries the cross-engine wait on cbfix_{k+1})
   -> x_k=E-Ei[w sA=ei]

DVE computes the tanh-bias prefix dots off the critical path:
cb_kk = b_kk + sum_{j<=kk-2} W[kk,j] x_j as an early prefix (mult+reduce
over wpy rows; steps kk<=NSPEC via tensor_scalar chains so the big wpy
DMA can land late) plus a one-op fix-up adding W[kk,kk-2]x_{kk-2}; the
W[kk,kk-1]x_{kk-1} term rides the tanh scale immediate.
SP issues both input DMAs at t=0 (HWDGE) and holds the pre-decoded output
DMA waiting on the last x.
"""

import numpy as np

B, D = 1024, 32
NCORES = 8
ROWS = B // NCORES  # 128 rows per core == SBUF partitions
NSPEC = 5           # prefix dots for kk=2..NSPEC via tensor_scalar chains


def _softplus64(x):
    x = x.astype(np.float64)
    return np.log1p(np.exp(-np.abs(x))) + np.maximum(x, 0)


def build(y, W, s, b):
    """Build the SPMD Bass program; returns (nc, in_maps)."""
    from contextlib import ExitStack
    import concourse.bass as bass
    from concourse import mybir

    f32 = mybir.dt.float32
    Alu = mybir.AluOpType
    Act = mybir.ActivationFunctionType

    y = np.ascontiguousarray(np.asarray(y), dtype=np.float32)
    W64 = np.asarray(W, dtype=np.float64)
    s64 = np.asarray(s, dtype=np.float64)
    b64 = np.asarray(b, dtype=np.float64)

    # ---- host precompute (elementwise input normalization only) ----
    abar = 10.0 * _softplus64(s64)
    sqrt_abar = np.sqrt(abar)
    kappa = 10.0 * abar ** -1.5
    CC = 3.0 * np.sqrt(3.0) / 2.0
    kz = (CC * kappa).astype(np.float32)
    Yz = (CC * 10.0 * y.astype(np.float64) * abar[None, :] ** -1.5).astype(np.float32)
    sat64 = sqrt_abar / np.sqrt(3.0)
    lnsat = np.log(sat64).astype(np.float32)
    Wq = W64.astype(np.float32)          # weights on x are original W
    c0 = float(-kz[0] * np.tanh(b64[0]))
    bt1 = float(b64[1])

    WPY = np.zeros((D - 2, D), np.float32)
    for k in range(2, D):
        WPY[k - 2, 0] = b64[k]
        WPY[k - 2, 1:k - 1] = Wq[k, 0:k - 2]
    WPYB = np.ascontiguousarray(np.broadcast_to(WPY[None], (ROWS, D - 2, D)))

    # hdr columns: [ yz (D) | lnsat (D) | one | c0 | bt1 | zero | b_2..b_NSPEC ]
    HW = 2 * D + 4 + (NSPEC - 1)
    LNSB = np.broadcast_to(lnsat[None, :], (ROWS, D))

    nc = bass.Bass()
    hd_d = nc.dram_tensor("hdr", [ROWS, HW], f32, kind="ExternalInput")
    wp_d = nc.dram_tensor("wpy", [ROWS, D - 2, D], f32, kind="ExternalInput")
    xo_d = nc.dram_tensor("xout", [ROWS, D], f32, kind="ExternalOutput")

    with ExitStack() as ctx:
        vx = ctx.enter_context(nc.sbuf_tensor([ROWS, D + 1], f32))  # [1, x_0..]
        hdr = ctx.enter_context(nc.sbuf_tensor([ROWS, HW], f32))
        yz = hdr[:, 0:D]
        lnsb = hdr[:, D:2 * D]
        onec = hdr[:, 2 * D:2 * D + 1]
        c0c = hdr[:, 2 * D + 1:2 * D + 2]
        bt1c = hdr[:, 2 * D + 2:2 * D + 3]
        bsc = hdr[:, 2 * D + 4:2 * D + 4 + (NSPEC - 1)]
        wpy = ctx.enter_context(nc.sbuf_tensor([ROWS, D - 2, D], f32))
        prod = ctx.enter_context(nc.sbuf_tensor([ROWS, D], f32))
        tt = ctx.enter_context(nc.sbuf_tensor([ROWS, 1], f32))
        z2 = ctx.enter_context(nc.sbuf_tensor([ROWS, 1], f32))
        rr = ctx.enter_context(nc.sbuf_tensor([ROWS, 1], f32))
        ll = ctx.enter_context(nc.sbuf_tensor([ROWS, 1], f32))
        ee = ctx.enter_context(nc.sbuf_tensor([ROWS, 1], f32))
        ei = ctx.enter_context(nc.sbuf_tensor([ROWS, 1], f32))
        zz = ctx.enter_context(nc.sbuf_tensor([ROWS, 1], f32))
        cb = ctx.enter_context(nc.sbuf_tensor([ROWS, 2], f32))
        cba = ctx.enter_context(nc.sbuf_tensor([ROWS, 2 + NSPEC - 2], f32))
        s_dma = ctx.enter_context(nc.semaphore("s_dma"))
        sA = ctx.enter_context(nc.semaphore("sA"))    # ACT chain counter
        sV = ctx.enter_context(nc.semaphore("sV"))    # DVE chain counter
        block = ctx.enter_context(nc.Block())

        # ---- pre-pass: compute every sem-count landmark ----
        a_tanh, a_z, a_sq, a_sqrt, a_ln, a_ei, a_sub = {}, {}, {}, {}, {}, {}, {}
        pa = 0
        for k in range(D):
            if k >= 1:
                pa += 1
                a_tanh[k] = pa
            pa += 1
            a_z[k] = pa
            pa += 1
            a_sq[k] = pa
            pa += 1
            a_sqrt[k] = pa
            pa += 1
            a_ln[k] = pa
            pa += 1  # E
            pa += 1
            a_ei[k] = pa
            pa += 1
            a_sub[k] = pa

        d_cbfix, d_cba, d_mult = {}, {}, {}
        pd = 1  # memset
        for k in range(D):
            kd = k + 3
            if NSPEC + 1 <= kd <= D - 1:
                pd += 1
                d_mult[kd] = pd
                pd += 1
                d_cba[kd] = pd
            kk = k + 2
            if kk <= D - 1:
                pd += 1
                d_cbfix[kk] = pd
            for kk2 in range(k + 3, NSPEC + 1):
                pd += 1
                d_cba[kk2] = pd

        @block.scalar
        def _(scalar):
            for k in range(D):
                # tanh_k (k=0: T_0=tanh(b_0) folded into c0)
                if k == 1:
                    nc.scalar.activation(
                        out=tt[:, :], in_=vx[:, 1:2], func=Act.Tanh,
                        bias=bt1c[:, :], scale=float(Wq[1, 0]))._wait_ge(
                            sA, a_sub[0]).then_inc(sA, 1)
                elif k >= 2:
                    nc.scalar.activation(
                        out=tt[:, :], in_=vx[:, k:k + 1], func=Act.Tanh,
                        bias=cb[:, k % 2:k % 2 + 1],
                        scale=float(Wq[k, k - 1]))._wait_ge(
                            sA, a_sub[k - 1]).then_inc(sA, 1)
                # z_k = -kz*T + yz (Ln's bias operand)
                if k == 0:
                    nc.scalar.activation(
                        out=zz[:, :], in_=yz[:, 0:1], func=Act.Identity,
                        bias=c0c[:, :], scale=1.0)._wait_ge(
                            s_dma, 16).then_inc(sA, 1)
                else:
                    nc.scalar.activation(
                        out=zz[:, :], in_=tt[:, :], func=Act.Identity,
                        bias=yz[:, k:k + 1], scale=float(-kz[k]))._wait_ge(
                            sA, a_tanh[k]).then_inc(sA, 1)
                # z2 = Square(-kz*T + yz): no wait (tanh retired via z's gate)
                if k == 0:
                    nc.scalar.activation(
                        out=z2[:, :], in_=yz[:, 0:1], func=Act.Square,
                        bias=c0c[:, :], scale=1.0).then_inc(sA, 1)
                else:
                    nc.scalar.activation(
                        out=z2[:, :], in_=tt[:, :], func=Act.Square,
                        bias=yz[:, k:k + 1],
                        scale=float(-kz[k])).then_inc(sA, 1)
                nc.scalar.activation(
                    out=rr[:, :], in_=z2[:, :], func=Act.Sqrt,
                    bias=onec[:, :], scale=1.0)._wait_ge(
                        sA, a_sq[k]).then_inc(sA, 1)
                nc.scalar.activation(
                    out=ll[:, :], in_=rr[:, :], func=Act.Ln, bias=zz[:, :],
                    scale=1.0)._wait_ge(sA, a_sqrt[k]).then_inc(sA, 1)
                nc.scalar.activation(
                    out=ee[:, :], in_=ll[:, :], func=Act.Exp,
                    bias=lnsb[:, k:k + 1], scale=float(1.0 / 3.0))._wait_ge(
                        sA, a_ln[k]).then_inc(sA, 1)
                # Ei: sA-ordering implied via E; free slot carries the
                # cross-engine gate on next step's tanh bias (cbfix_{k+1})
                inst = nc.scalar.activation(
                    out=ei[:, :], in_=ll[:, :], func=Act.Exp,
                    bias=lnsb[:, k:k + 1], scale=float(-1.0 / 3.0))
                if k + 1 in d_cbfix:
                    inst._wait_ge(sV, d_cbfix[k + 1])
                inst.then_inc(sA, 1)
                nc.scalar.activation(
                    out=vx[:, k + 1:k + 2], in_=ei[:, :], func=Act.Identity,
                    bias=ee[:, :], scale=-1.0)._wait_ge(
                        sA, a_ei[k]).then_inc(sA, 1)

        @block.vector
        def _(vector):
            # memset also gates wpy readiness for every later wpy read
            nc.vector.memset(vx[:, 0:1], 1.0)._wait_ge(
                s_dma, 32).then_inc(sV, 1)
            for k in range(D):
                kd = k + 3
                if NSPEC + 1 <= kd <= D - 1:
                    nc.vector.tensor_tensor(
                        out=prod[:, 0:kd - 1], in0=vx[:, 0:kd - 1],
                        in1=wpy[:, kd - 2, 0:kd - 1],
                        op=Alu.mult)._wait_ge(sA, a_sub[k]).then_inc(sV, 1)
                    nc.vector.tensor_reduce(
                        out=cba[:, kd % 2:kd % 2 + 1], in_=prod[:, 0:kd - 1],
                        axis=mybir.AxisListType.X, op=Alu.add)._wait_ge(
                            sV, d_mult[kd]).then_inc(sV, 1)
                kk = k + 2
                if kk <= D - 1:
                    if kk == 2:
                        cba_src = bsc[:, 0:1]
                    elif kk <= NSPEC:
                        cba_src = cba[:, kk - 1:kk]
                    else:
                        cba_src = cba[:, kk % 2:kk % 2 + 1]
                    inst = nc.vector.tensor_scalar(
                        out=cb[:, kk % 2:kk % 2 + 1], in0=vx[:, k + 1:k + 2],
                        scalar1=float(Wq[kk, kk - 2]), scalar2=cba_src,
                        op0=Alu.mult, op1=Alu.add)
                    if kd in d_mult:
                        # x_k retirement implied by this iteration's mult;
                        # free slot carries the cba RAW edge
                        inst._wait_ge(sV, d_cba[kk])
                    else:
                        inst._wait_ge(sA, a_sub[k])
                    inst.then_inc(sV, 1)
                # special prefix chains (kk2=3..NSPEC): add Wq[kk2,k]*x_k.
                # No waits: this iteration's cbfix already gated on
                # a_sub[k], and the previous partial ran a full step ago.
                for kk2 in range(k + 3, NSPEC + 1):
                    src = bsc[:, kk2 - 2:kk2 - 1] if k == 0 else cba[:, kk2 - 1:kk2]
                    nc.vector.tensor_scalar(
                        out=cba[:, kk2 - 1:kk2], in0=vx[:, k + 1:k + 2],
                        scalar1=float(Wq[kk2, k]), scalar2=src,
                        op0=Alu.mult, op1=Alu.add).then_inc(sV, 1)

        @block.sync
        def _(sync):
            sync.dma_start(out=hdr[:, :], in_=hd_d[:, :]).then_inc(s_dma, 16)
            sync.dma_start(out=wpy[:, :, :], in_=wp_d[:, :, :]).then_inc(s_dma, 16)
            sync.dma_start(out=xo_d[:, :], in_=vx[:, 1:D + 1])._wait_ge(
                sA, a_sub[D - 1]).then_inc(s_dma, 16)
            sync.wait_ge(s_dma, 48)

    bs_cols = np.broadcast_to(
        b64[2:NSPEC + 1].astype(np.float32)[None, :], (ROWS, NSPEC - 1))
    in_maps = []
    for c in range(NCORES):
        hdr_np = np.concatenate([
            Yz[c * ROWS:(c + 1) * ROWS],
            LNSB,
            np.full((ROWS, 1), 1.0, np.float32),
            np.full((ROWS, 1), c0, np.float32),
            np.full((ROWS, 1), bt1, np.float32),
            np.zeros((ROWS, 1), np.float32),
            bs_cols,
        ], axis=1)
        in_maps.append({"hdr": np.ascontiguousarray(hdr_np), "wpy": WPYB})
    return nc, in_maps


def kernel(y, W, s, b):
    from concourse.bass_utils import run_bass_kernel_spmd

    nc, in_maps = build(y, W, s, b)
    res = run_bass_kernel_spmd(nc, in_maps, list(range(NCORES))).results
    X = np.concatenate([res[c]["xout"] for c in range(NCORES)], axis=0)
    return X.astype(np.float32)


if __name__ == "__main__":
    rng = np.random.default_rng(0)
    y = rng.standard_normal((B, D)).astype(np.float32)
    W = np.tril(rng.standard_normal((D, D)), -1).astype(np.float32) * 0.5
    s = rng.standard_normal(D).astype(np.float32)
    b = rng.standard_normal(D).astype(np.float32)
    X = kernel(y=y, W=W, s=s, b=b)
    print("out", X.shape, X.dtype, X[0, :4])


# revision 4
# speedup vs baseline: 1.2900x; 1.1154x over previous
"""Trainium2 Bass kernel for nn_AutoregressiveBisectionInverter (v10).

Closed-form cubic root per autoregressive step: solve v^3+v = nd via
v = (t^(1/3) - t^(-1/3))/sqrt(3), t = z + sqrt(z^2+1), z = (3sqrt3/2)nd;
x_k = sat_k * v. With E' = sat*e^(ln(t)/3) (sat folded into Exp's bias),
x_k = E' - sat^2/E', so the second Exp of earlier versions becomes a DVE
reciprocal + one fused tensor_scalar.

Drain-free semaphore-edge schedule, one wait per instruction, transitive
coverage through the in-order engines:

ACT (6 ops/step, 342ns issue):
  tanh[w sV=cbfix_k]
  -> z=Identity[w sA=tanh_k] -> Square(no wait; z's gate covers tanh)
  -> Sqrt[w sA=sq] -> Ln(bias=z)[w sA=sqrt] -> E'=Exp(L/3+lnsat)[w sA=ln]

DVE per iteration k: rcp=1/E' [w sA=E_k, transitively covers the whole
step] -> x_k = -sat^2*rcp + E' [w sV=rcp] -> cbfix_{k+1} =
W[k+1,k-1]x_{k-1}+cba [w sV=its cba producer, a formal RAW edge past the
wide-op ack window] -> paired prefix dots (rows kk,kk+1 in one [128,2,m]
mult+reduce) -> special tensor_scalar chains for the first rows.
tanh_k waits d_cbfix[k], which transitively covers x_{k-1}'s retirement.
"""

import numpy as np

B, D = 1024, 32
NCORES = 8
ROWS = B // NCORES  # 128 rows per core == SBUF partitions
NSPEC = 5           # prefix dots for kk=2..NSPEC via tensor_scalar chains


def _softplus64(x):
    x = x.astype(np.float64)
    return np.log1p(np.exp(-np.abs(x))) + np.maximum(x, 0)


def build(y, W, s, b):
    """Build the SPMD Bass program; returns (nc, in_maps)."""
    from contextlib import ExitStack
    import concourse.bass as bass
    from concourse import mybir

    f32 = mybir.dt.float32
    Alu = mybir.AluOpType
    Act = mybir.ActivationFunctionType

    y = np.ascontiguousarray(np.asarray(y), dtype=np.float32)
    W64 = np.asarray(W, dtype=np.float64)
    s64 = np.asarray(s, dtype=np.float64)
    b64 = np.asarray(b, dtype=np.float64)

    # ---- host precompute (elementwise input normalization only) ----
    abar = 10.0 * _softplus64(s64)
    sqrt_abar = np.sqrt(abar)
    kappa = 10.0 * abar ** -1.5
    CC = 3.0 * np.sqrt(3.0) / 2.0
    kz = (CC * kappa).astype(np.float32)
    Yz = (CC * 10.0 * y.astype(np.float64) * abar[None, :] ** -1.5).astype(np.float32)
    sat64 = sqrt_abar / np.sqrt(3.0)
    lnsat = np.log(sat64).astype(np.float32)
    sat2 = (sat64 * sat64).astype(np.float32)
    Wq = W64.astype(np.float32)          # weights on x are original W
    c0 = float(-kz[0] * np.tanh(b64[0]))
    bt1 = float(b64[1])

    # wpy row for kk holds [b_kk, W[kk,0..kk-3], 0...]; rows kk-2 and kk-1
    # are sliced together for the paired dots.
    WPY = np.zeros((D - 2, D), np.float32)
    for k in range(2, D):
        WPY[k - 2, 0] = b64[k]
        WPY[k - 2, 1:k - 1] = Wq[k, 0:k - 2]
    WPYB = np.ascontiguousarray(np.broadcast_to(WPY[None], (ROWS, D - 2, D)))

    # hdr columns: [ yz (D) | lnsat (D) | one | c0 | bt1 | b_2..b_NSPEC ]
    HW = 2 * D + 3 + (NSPEC - 1)
    LNSB = np.broadcast_to(lnsat[None, :], (ROWS, D))

    nc = bass.Bass()
    hd_d = nc.dram_tensor("hdr", [ROWS, HW], f32, kind="ExternalInput")
    wp_d = nc.dram_tensor("wpy", [ROWS, D - 2, D], f32, kind="ExternalInput")
    xo_d = nc.dram_tensor("xout", [ROWS, D], f32, kind="ExternalOutput")

    with ExitStack() as ctx:
        vx = ctx.enter_context(nc.sbuf_tensor([ROWS, D + 1], f32))  # [1, x_0..]
        hdr = ctx.enter_context(nc.sbuf_tensor([ROWS, HW], f32))
        yz = hdr[:, 0:D]
        lnsb = hdr[:, D:2 * D]
        onec = hdr[:, 2 * D:2 * D + 1]
        c0c = hdr[:, 2 * D + 1:2 * D + 2]
        bt1c = hdr[:, 2 * D + 2:2 * D + 3]
        bsc = hdr[:, 2 * D + 3:2 * D + 3 + (NSPEC - 1)]
        wpy = ctx.enter_context(nc.sbuf_tensor([ROWS, D - 2, D], f32))
        prod = ctx.enter_context(nc.sbuf_tensor([ROWS, 2, D], f32))
        tt = ctx.enter_context(nc.sbuf_tensor([ROWS, 1], f32))
        z2 = ctx.enter_context(nc.sbuf_tensor([ROWS, 1], f32))
        rr = ctx.enter_context(nc.sbuf_tensor([ROWS, 1], f32))
        ll = ctx.enter_context(nc.sbuf_tensor([ROWS, 1], f32))
        ee = ctx.enter_context(nc.sbuf_tensor([ROWS, 1], f32))
        rcp = ctx.enter_context(nc.sbuf_tensor([ROWS, 1], f32))
        zz = ctx.enter_context(nc.sbuf_tensor([ROWS, 1], f32))
        cb = ctx.enter_context(nc.sbuf_tensor([ROWS, 2], f32))
        # cba: [0:4] two double-col buffers for paired dots (by pair parity);
        # [4:4+NSPEC-2] dedicated cols for the special chains kk=3..NSPEC
        cba = ctx.enter_context(nc.sbuf_tensor([ROWS, 4 + NSPEC - 2], f32))
        s_dma = ctx.enter_context(nc.semaphore("s_dma"))
        sA = ctx.enter_context(nc.semaphore("sA"))    # ACT chain counter
        sV = ctx.enter_context(nc.semaphore("sV"))    # DVE chain counter
        block = ctx.enter_context(nc.Block())

        # ---- pre-pass: compute every sem-count landmark ----
        a_tanh, a_sq, a_sqrt, a_ln, a_e = {}, {}, {}, {}, {}
        pa = 0
        for k in range(D):
            if k >= 1:
                pa += 1
                a_tanh[k] = pa
            pa += 1  # z
            pa += 1
            a_sq[k] = pa
            pa += 1
            a_sqrt[k] = pa
            pa += 1
            a_ln[k] = pa
            pa += 1
            a_e[k] = pa

        # DVE emission order per iteration k: [wpy-gate nop at k=4]
        #   rcp_k | subx_k | cbfix_{k+1} | pair(kk=k+2) | specials
        d_sub, d_rcp, d_red, d_cbfix, d_spec = {}, {}, {}, {}, {}
        pd = 1  # memset
        for k in range(D):
            if k == 4:
                pd += 1  # wpy-gate nop
            pd += 1
            d_rcp[k] = pd
            pd += 1
            d_sub[k] = pd
            if 2 <= k + 1 <= D - 1:
                pd += 1  # cbfix_{k+1}
                d_cbfix[k + 1] = pd
            kk = k + 2
            if kk >= NSPEC + 1 and kk % 2 == 0 and kk <= D - 2:
                pd += 1  # mult2
                pd += 1
                d_red[kk] = pd
            for kk2 in range(k + 3, NSPEC + 1):
                pd += 1
                d_spec[kk2] = pd

        @block.scalar
        def _(scalar):
            for k in range(D):
                # tanh_k (k=0: T_0=tanh(b_0) folded into c0)
                if k == 1:
                    nc.scalar.activation(
                        out=tt[:, :], in_=vx[:, 1:2], func=Act.Tanh,
                        bias=bt1c[:, :], scale=float(Wq[1, 0]))._wait_ge(
                            sV, d_sub[0]).then_inc(sA, 1)
                elif k >= 2:
                    nc.scalar.activation(
                        out=tt[:, :], in_=vx[:, k:k + 1], func=Act.Tanh,
                        bias=cb[:, k % 2:k % 2 + 1],
                        scale=float(Wq[k, k - 1]))._wait_ge(
                            sV, d_cbfix[k]).then_inc(sA, 1)
                # z_k = -kz*T + yz (Ln's bias operand)
                if k == 0:
                    nc.scalar.activation(
                        out=zz[:, :], in_=yz[:, 0:1], func=Act.Identity,
                        bias=c0c[:, :], scale=1.0)._wait_ge(
                            s_dma, 16).then_inc(sA, 1)
                else:
                    nc.scalar.activation(
                        out=zz[:, :], in_=tt[:, :], func=Act.Identity,
                        bias=yz[:, k:k + 1], scale=float(-kz[k]))._wait_ge(
                            sA, a_tanh[k]).then_inc(sA, 1)
                # Square: no wait (z's gate covers tanh's retirement)
                if k == 0:
                    nc.scalar.activation(
                        out=z2[:, :], in_=yz[:, 0:1], func=Act.Square,
                        bias=c0c[:, :], scale=1.0).then_inc(sA, 1)
                else:
                    nc.scalar.activation(
                        out=z2[:, :], in_=tt[:, :], func=Act.Square,
                        bias=yz[:, k:k + 1],
                        scale=float(-kz[k])).then_inc(sA, 1)
                nc.scalar.activation(
                    out=rr[:, :], in_=z2[:, :], func=Act.Sqrt,
                    bias=onec[:, :], scale=1.0)._wait_ge(
                        sA, a_sq[k]).then_inc(sA, 1)
                nc.scalar.activation(
                    out=ll[:, :], in_=rr[:, :], func=Act.Ln, bias=zz[:, :],
                    scale=1.0)._wait_ge(sA, a_sqrt[k]).then_inc(sA, 1)
                nc.scalar.activation(
                    out=ee[:, :], in_=ll[:, :], func=Act.Exp,
                    bias=lnsb[:, k:k + 1], scale=float(1.0 / 3.0))._wait_ge(
                        sA, a_ln[k]).then_inc(sA, 1)

        @block.vector
        def _(vector):
            nc.vector.memset(vx[:, 0:1], 1.0).then_inc(sV, 1)
            for k in range(D):
                if k == 4:
                    # wpy-gate: everything after this sees the wpy DMA done
                    nc.vector.memset(cba[:, 0:1], 0.0)._wait_ge(
                        s_dma, 32).then_inc(sV, 1)
                # rcp_k = 1/E'  (its wait transitively implies everything
                # through tanh_k, incl. subx_{k-1} and cbfix_k)
                nc.vector.reciprocal(out=rcp[:, :], in_=ee[:, :])._wait_ge(
                    sA, a_e[k]).then_inc(sV, 1)
                # x_k = -sat^2*rcp + E'
                nc.vector.tensor_scalar(
                    out=vx[:, k + 1:k + 2], in0=rcp[:, :],
                    scalar1=float(-sat2[k]), scalar2=ee[:, 0:1],
                    op0=Alu.mult, op1=Alu.add)._wait_ge(
                        sV, d_rcp[k]).then_inc(sV, 1)
                # cbfix_{k+1}: cb = W[k+1,k-1]*x_{k-1} + cba  (k+1 in 2..D-1)
                # single wait = its cba producer (RAW edge past the wide-op
                # ack window); x_{k-1} covered transitively via rcp's wait.
                kk1 = k + 1
                if 2 <= kk1 <= D - 1:
                    if kk1 == 2:
                        cba_src = bsc[:, 0:1]
                        w_cba = None
                    elif kk1 <= NSPEC:
                        cba_src = cba[:, kk1 + 1:kk1 + 2]  # special col
                        w_cba = d_spec[kk1]
                    else:
                        cp = 2 * ((kk1 // 2) % 2)
                        cba_src = cba[:, cp + (kk1 % 2):cp + (kk1 % 2) + 1]
                        w_cba = d_red[kk1] if kk1 % 2 == 0 else d_red[kk1 - 1]
                    inst = nc.vector.tensor_scalar(
                        out=cb[:, kk1 % 2:kk1 % 2 + 1], in0=vx[:, k:k + 1],
                        scalar1=float(Wq[kk1, kk1 - 2]), scalar2=cba_src,
                        op0=Alu.mult, op1=Alu.add)
                    if w_cba is not None:
                        inst._wait_ge(sV, w_cba)
                    inst.then_inc(sV, 1)
                kk = k + 2
                if kk >= NSPEC + 1 and kk % 2 == 0 and kk <= D - 2:
                    m = kk
                    c4 = 2 * ((kk // 2) % 2)
                    a = vx[:, 0:m]
                    vxb = bass.AP(tensor=a.tensor, offset=a.offset,
                                  ap=[list(a.ap[0]), [0, 2], [1, m]])
                    nc.vector.tensor_tensor(
                        out=prod[:, 0:2, 0:m], in0=vxb,
                        in1=wpy[:, kk - 2:kk, 0:m], op=Alu.mult)._wait_ge(
                            sV, d_sub[k]).then_inc(sV, 1)
                    nc.vector.tensor_reduce(
                        out=cba[:, c4:c4 + 2], in_=prod[:, 0:2, 0:m],
                        axis=mybir.AxisListType.X, op=Alu.add)._wait_ge(
                            sV, d_red[kk] - 1).then_inc(sV, 1)
                # special prefix chains (kk2=3..NSPEC): add Wq[kk2,k]*x_k
                first = True
                for kk2 in range(k + 3, NSPEC + 1):
                    src = bsc[:, kk2 - 2:kk2 - 1] if k == 0 else cba[:, kk2 + 1:kk2 + 2]
                    inst = nc.vector.tensor_scalar(
                        out=cba[:, kk2 + 1:kk2 + 2], in0=vx[:, k + 1:k + 2],
                        scalar1=float(Wq[kk2, k]), scalar2=src,
                        op0=Alu.mult, op1=Alu.add)
                    if first:
                        inst._wait_ge(sV, d_sub[k])  # x_k RAW edge
                        first = False
                    inst.then_inc(sV, 1)

        @block.sync
        def _(sync):
            sync.dma_start(out=hdr[:, :], in_=hd_d[:, :]).then_inc(s_dma, 16)
            sync.dma_start(out=wpy[:, :, :], in_=wp_d[:, :, :]).then_inc(s_dma, 16)
            sync.dma_start(out=xo_d[:, :], in_=vx[:, 1:D + 1])._wait_ge(
                sV, d_sub[D - 1]).then_inc(s_dma, 16)
            sync.wait_ge(s_dma, 48)

    bs_cols = np.broadcast_to(
        b64[2:NSPEC + 1].astype(np.float32)[None, :], (ROWS, NSPEC - 1))
    in_maps = []
    for c in range(NCORES):
        hdr_np = np.concatenate([
            Yz[c * ROWS:(c + 1) * ROWS],
            LNSB,
            np.full((ROWS, 1), 1.0, np.float32),
            np.full((ROWS, 1), c0, np.float32),
            np.full((ROWS, 1), bt1, np.float32),
            bs_cols,
        ], axis=1)
        in_maps.append({"hdr": np.ascontiguousarray(hdr_np), "wpy": WPYB})
    return nc, in_maps


def kernel(y, W, s, b):
    from concourse.bass_utils import run_bass_kernel_spmd

    nc, in_maps = build(y, W, s, b)
    res = run_bass_kernel_spmd(nc, in_maps, list(range(NCORES))).results
    X = np.concatenate([res[c]["xout"] for c in range(NCORES)], axis=0)
    return X.astype(np.float32)


if __name__ == "__main__":
    rng = np.random.default_rng(0)
    y = rng.standard_normal((B, D)).astype(np.float32)
    W = np.tril(rng.standard_normal((32, 32)), -1).astype(np.float32) * 0.5
    s = rng.standard_normal(D).astype(np.float32)
    b = rng.standard_normal(D).astype(np.float32)
    X = kernel(y=y, W=W, s=s, b=b)
    print("out", X.shape, X.dtype, X[0, :4])


# revision 5
# speedup vs baseline: 1.2976x; 1.0059x over previous
"""Trainium2 Bass kernel for nn_AutoregressiveBisectionInverter (v10).

Closed-form cubic root per autoregressive step: solve v^3+v = nd via
v = (t^(1/3) - t^(-1/3))/sqrt(3), t = z + sqrt(z^2+1), z = (3sqrt3/2)nd;
x_k = sat_k * v. With E' = sat*e^(ln(t)/3) (sat folded into Exp's bias),
x_k = E' - sat^2/E', so the second Exp of earlier versions becomes a DVE
reciprocal + one fused tensor_scalar.

Drain-free semaphore-edge schedule, one wait per instruction, transitive
coverage through the in-order engines:

ACT (6 ops/step, 342ns issue):
  tanh[w sV=cbfix_k]
  -> z=Identity[w sA=tanh_k] -> Square(no wait; z's gate covers tanh)
  -> Sqrt[w sA=sq] -> Ln(bias=z)[w sA=sqrt] -> E'=Exp(L/3+lnsat)[w sA=ln]

DVE per iteration k: rcp=1/E' [w sA=E_k, transitively covers the whole
step] -> x_k = -sat^2*rcp + E' [w sV=rcp] -> cbfix_{k+1} =
W[k+1,k-1]x_{k-1}+cba [w sV=its cba producer, a formal RAW edge past the
wide-op ack window] -> paired prefix dots (rows kk,kk+1 in one [128,2,m]
mult+reduce) -> special tensor_scalar chains for the first rows.
tanh_k waits d_cbfix[k], which transitively covers x_{k-1}'s retirement.
"""

import numpy as np

B, D = 1024, 32
NCORES = 8
ROWS = B // NCORES  # 128 rows per core == SBUF partitions
NSPEC = 5           # prefix dots for kk=2..NSPEC via tensor_scalar chains


def _softplus64(x):
    x = x.astype(np.float64)
    return np.log1p(np.exp(-np.abs(x))) + np.maximum(x, 0)


def build(y, W, s, b):
    """Build the SPMD Bass program; returns (nc, in_maps)."""
    from contextlib import ExitStack
    import concourse.bass as bass
    from concourse import mybir

    f32 = mybir.dt.float32
    Alu = mybir.AluOpType
    Act = mybir.ActivationFunctionType

    y = np.ascontiguousarray(np.asarray(y), dtype=np.float32)
    W64 = np.asarray(W, dtype=np.float64)
    s64 = np.asarray(s, dtype=np.float64)
    b64 = np.asarray(b, dtype=np.float64)

    # ---- host precompute (elementwise input normalization only) ----
    abar = 10.0 * _softplus64(s64)
    sqrt_abar = np.sqrt(abar)
    kappa = 10.0 * abar ** -1.5
    CC = 3.0 * np.sqrt(3.0) / 2.0
    kz = (CC * kappa).astype(np.float32)
    Yz = (CC * 10.0 * y.astype(np.float64) * abar[None, :] ** -1.5).astype(np.float32)
    sat64 = sqrt_abar / np.sqrt(3.0)
    lnsat = np.log(sat64).astype(np.float32)
    sat2 = (sat64 * sat64).astype(np.float32)
    Wq = W64.astype(np.float32)          # weights on x are original W
    c0 = float(-kz[0] * np.tanh(b64[0]))
    bt1 = float(b64[1])

    # wpy row for kk holds [b_kk, W[kk,0..kk-3], 0...]; rows kk-2 and kk-1
    # are sliced together for the paired dots. Only rows kk=6..D-1 are read
    # (the specials cover kk<=NSPEC), so the table starts at row 4.
    NWPY = D - 2 - 4
    WPY = np.zeros((NWPY, D), np.float32)
    for k in range(6, D):
        WPY[k - 6, 0] = b64[k]
        WPY[k - 6, 1:k - 1] = Wq[k, 0:k - 2]
    WPYB = np.ascontiguousarray(np.broadcast_to(WPY[None], (ROWS, NWPY, D)))

    # hdr columns: [ one | c0 | bt1 | b_2..b_NSPEC | yzA(NA) | lnsA(NA) |
    #                yzB | lnsB ]  (A-part lands in the first small DMA)
    NA = 10
    NCST = 3 + (NSPEC - 1)
    HW = NCST + 2 * D
    LNSB = np.broadcast_to(lnsat[None, :], (ROWS, D))

    nc = bass.Bass()
    hd_d = nc.dram_tensor("hdr", [ROWS, HW], f32, kind="ExternalInput")
    wp_d = nc.dram_tensor("wpy", [ROWS, NWPY, D], f32, kind="ExternalInput")
    xo_d = nc.dram_tensor("xout", [ROWS, D], f32, kind="ExternalOutput")

    with ExitStack() as ctx:
        vx = ctx.enter_context(nc.sbuf_tensor([ROWS, D + 1], f32))  # [1, x_0..]
        hdr = ctx.enter_context(nc.sbuf_tensor([ROWS, HW], f32))
        onec = hdr[:, 0:1]
        c0c = hdr[:, 1:2]
        bt1c = hdr[:, 2:3]
        bsc = hdr[:, 3:3 + (NSPEC - 1)]

        def yzc(k):
            c = NCST + k if k < NA else NCST + 2 * NA + (k - NA)
            return hdr[:, c:c + 1]

        def lnsc(k):
            c = NCST + NA + k if k < NA else NCST + NA + D + (k - NA)
            return hdr[:, c:c + 1]

        wpy = ctx.enter_context(nc.sbuf_tensor([ROWS, NWPY, D], f32))
        prod = ctx.enter_context(nc.sbuf_tensor([ROWS, 2, D], f32))
        tt = ctx.enter_context(nc.sbuf_tensor([ROWS, 1], f32))
        z2 = ctx.enter_context(nc.sbuf_tensor([ROWS, 1], f32))
        rr = ctx.enter_context(nc.sbuf_tensor([ROWS, 1], f32))
        ll = ctx.enter_context(nc.sbuf_tensor([ROWS, 1], f32))
        ee = ctx.enter_context(nc.sbuf_tensor([ROWS, 1], f32))
        rcp = ctx.enter_context(nc.sbuf_tensor([ROWS, 1], f32))
        zz = ctx.enter_context(nc.sbuf_tensor([ROWS, 1], f32))
        cb = ctx.enter_context(nc.sbuf_tensor([ROWS, 2], f32))
        # cba: [0:4] two double-col buffers for paired dots (by pair parity);
        # [4:4+NSPEC-2] dedicated cols for the special chains kk=3..NSPEC
        cba = ctx.enter_context(nc.sbuf_tensor([ROWS, 4 + NSPEC - 2], f32))
        s_dma = ctx.enter_context(nc.semaphore("s_dma"))
        sA = ctx.enter_context(nc.semaphore("sA"))    # ACT chain counter
        sV = ctx.enter_context(nc.semaphore("sV"))    # DVE chain counter
        block = ctx.enter_context(nc.Block())

        # ---- pre-pass: compute every sem-count landmark ----
        a_tanh, a_sq, a_sqrt, a_ln, a_e = {}, {}, {}, {}, {}
        pa = 0
        for k in range(D):
            if k >= 1:
                pa += 1
                a_tanh[k] = pa
            pa += 1  # z
            pa += 1
            a_sq[k] = pa
            pa += 1
            a_sqrt[k] = pa
            pa += 1
            a_ln[k] = pa
            pa += 1
            a_e[k] = pa

        # DVE emission order per iteration k: [wpy-gate nop at k=4]
        #   rcp_k | subx_k | cbfix_{k+1} | pair(kk=k+2) | specials
        d_sub, d_rcp, d_red, d_cbfix, d_spec = {}, {}, {}, {}, {}
        pd = 1  # memset
        for k in range(D):
            if k == 4:
                pd += 1  # wpyA-gate nop
            if k == 13:
                pd += 1  # wpyB-gate nop
            pd += 1
            d_rcp[k] = pd
            pd += 1
            d_sub[k] = pd
            if 2 <= k + 1 <= D - 1:
                pd += 1  # cbfix_{k+1}
                d_cbfix[k + 1] = pd
            kk = k + 2
            if kk >= NSPEC + 1 and kk % 2 == 0 and kk <= D - 2:
                pd += 1  # mult2
                pd += 1
                d_red[kk] = pd
            for kk2 in range(k + 3, NSPEC + 1):
                pd += 1
                d_spec[kk2] = pd

        @block.scalar
        def _(scalar):
            for k in range(D):
                # tanh_k (k=0: T_0=tanh(b_0) folded into c0)
                if k == 1:
                    nc.scalar.activation(
                        out=tt[:, :], in_=vx[:, 1:2], func=Act.Tanh,
                        bias=bt1c[:, :], scale=float(Wq[1, 0]))._wait_ge(
                            sV, d_sub[0]).then_inc(sA, 1)
                elif k >= 2:
                    nc.scalar.activation(
                        out=tt[:, :], in_=vx[:, k:k + 1], func=Act.Tanh,
                        bias=cb[:, k % 2:k % 2 + 1],
                        scale=float(Wq[k, k - 1]))._wait_ge(
                            sV, d_cbfix[k]).then_inc(sA, 1)
                # z_k = -kz*T + yz (Ln's bias operand)
                if k == 0:
                    nc.scalar.activation(
                        out=zz[:, :], in_=yzc(0), func=Act.Identity,
                        bias=c0c[:, :], scale=1.0)._wait_ge(
                            s_dma, 16).then_inc(sA, 1)
                else:
                    nc.scalar.activation(
                        out=zz[:, :], in_=tt[:, :], func=Act.Identity,
                        bias=yzc(k), scale=float(-kz[k]))._wait_ge(
                            sA, a_tanh[k]).then_inc(sA, 1)
                # Square: no wait (z's gate covers tanh's retirement)
                if k == 0:
                    nc.scalar.activation(
                        out=z2[:, :], in_=yzc(0), func=Act.Square,
                        bias=c0c[:, :], scale=1.0).then_inc(sA, 1)
                else:
                    inst = nc.scalar.activation(
                        out=z2[:, :], in_=tt[:, :], func=Act.Square,
                        bias=yzc(k), scale=float(-kz[k]))
                    if k == NA - 1:
                        # hdrB (3rd DMA) gate: every later ACT op sees the
                        # yzB/lnsB columns; lands ~2.5us before this fires
                        inst._wait_ge(s_dma, 48)
                    inst.then_inc(sA, 1)
                nc.scalar.activation(
                    out=rr[:, :], in_=z2[:, :], func=Act.Sqrt,
                    bias=onec[:, :], scale=1.0)._wait_ge(
                        sA, a_sq[k]).then_inc(sA, 1)
                nc.scalar.activation(
                    out=ll[:, :], in_=rr[:, :], func=Act.Ln, bias=zz[:, :],
                    scale=1.0)._wait_ge(sA, a_sqrt[k]).then_inc(sA, 1)
                nc.scalar.activation(
                    out=ee[:, :], in_=ll[:, :], func=Act.Exp,
                    bias=lnsc(k), scale=float(1.0 / 3.0))._wait_ge(
                        sA, a_ln[k]).then_inc(sA, 1)

        @block.vector
        def _(vector):
            nc.vector.memset(vx[:, 0:1], 1.0).then_inc(sV, 1)
            for k in range(D):
                if k == 4:
                    # wpyA-gate: pairs kk=6..14 see the wpyA DMA done
                    nc.vector.memset(cba[:, 0:1], 0.0)._wait_ge(
                        s_dma, 32).then_inc(sV, 1)
                if k == 13:
                    # wpyB-gate: pairs kk>=16 see the wpyB DMA done
                    nc.vector.memset(cba[:, 0:1], 0.0)._wait_ge(
                        s_dma, 64).then_inc(sV, 1)
                # rcp_k = 1/E'  (its wait transitively implies everything
                # through tanh_k, incl. subx_{k-1} and cbfix_k)
                nc.vector.reciprocal(out=rcp[:, :], in_=ee[:, :])._wait_ge(
                    sA, a_e[k]).then_inc(sV, 1)
                # x_k = -sat^2*rcp + E'
                nc.vector.tensor_scalar(
                    out=vx[:, k + 1:k + 2], in0=rcp[:, :],
                    scalar1=float(-sat2[k]), scalar2=ee[:, 0:1],
                    op0=Alu.mult, op1=Alu.add)._wait_ge(
                        sV, d_rcp[k]).then_inc(sV, 1)
                # cbfix_{k+1}: cb = W[k+1,k-1]*x_{k-1} + cba  (k+1 in 2..D-1)
                # single wait = its cba producer (RAW edge past the wide-op
                # ack window); x_{k-1} covered transitively via rcp's wait.
                kk1 = k + 1
                if 2 <= kk1 <= D - 1:
                    if kk1 == 2:
                        cba_src = bsc[:, 0:1]
                        w_cba = None
                    elif kk1 <= NSPEC:
                        cba_src = cba[:, kk1 + 1:kk1 + 2]  # special col
                        w_cba = d_spec[kk1]
                    else:
                        cp = 2 * ((kk1 // 2) % 2)
                        cba_src = cba[:, cp + (kk1 % 2):cp + (kk1 % 2) + 1]
                        w_cba = d_red[kk1] if kk1 % 2 == 0 else d_red[kk1 - 1]
                    inst = nc.vector.tensor_scalar(
                        out=cb[:, kk1 % 2:kk1 % 2 + 1], in0=vx[:, k:k + 1],
                        scalar1=float(Wq[kk1, kk1 - 2]), scalar2=cba_src,
                        op0=Alu.mult, op1=Alu.add)
                    if w_cba is not None:
                        inst._wait_ge(sV, w_cba)
                    inst.then_inc(sV, 1)
                kk = k + 2
                if kk >= NSPEC + 1 and kk % 2 == 0 and kk <= D - 2:
                    m = kk
                    c4 = 2 * ((kk // 2) % 2)
                    a = vx[:, 0:m]
                    vxb = bass.AP(tensor=a.tensor, offset=a.offset,
                                  ap=[list(a.ap[0]), [0, 2], [1, m]])
                    nc.vector.tensor_tensor(
                        out=prod[:, 0:2, 0:m], in0=vxb,
                        in1=wpy[:, kk - 6:kk - 4, 0:m], op=Alu.mult)._wait_ge(
                            sV, d_sub[k]).then_inc(sV, 1)
                    nc.vector.tensor_reduce(
                        out=cba[:, c4:c4 + 2], in_=prod[:, 0:2, 0:m],
                        axis=mybir.AxisListType.X, op=Alu.add)._wait_ge(
                            sV, d_red[kk] - 1).then_inc(sV, 1)
                # special prefix chains (kk2=3..NSPEC): add Wq[kk2,k]*x_k
                first = True
                for kk2 in range(k + 3, NSPEC + 1):
                    src = bsc[:, kk2 - 2:kk2 - 1] if k == 0 else cba[:, kk2 + 1:kk2 + 2]
                    inst = nc.vector.tensor_scalar(
                        out=cba[:, kk2 + 1:kk2 + 2], in0=vx[:, k + 1:k + 2],
                        scalar1=float(Wq[kk2, k]), scalar2=src,
                        op0=Alu.mult, op1=Alu.add)
                    if first:
                        inst._wait_ge(sV, d_sub[k])  # x_k RAW edge
                        first = False
                    inst.then_inc(sV, 1)

        NHA = NCST + 2 * NA        # hdrA: consts + yzA + lnsA
        NWA = 10                   # wpyA rows 0..9 (pairs kk=6..14)

        @block.sync
        def _(sync):
            sync.dma_start(out=hdr[:, 0:NHA],
                           in_=hd_d[:, 0:NHA]).then_inc(s_dma, 16)
            sync.dma_start(out=wpy[:, 0:NWA, :],
                           in_=wp_d[:, 0:NWA, :]).then_inc(s_dma, 16)
            sync.dma_start(out=hdr[:, NHA:HW],
                           in_=hd_d[:, NHA:HW]).then_inc(s_dma, 16)
            sync.dma_start(out=wpy[:, NWA:NWPY, :],
                           in_=wp_d[:, NWA:NWPY, :]).then_inc(s_dma, 16)
            sync.dma_start(out=xo_d[:, :], in_=vx[:, 1:D + 1])._wait_ge(
                sV, d_sub[D - 1]).then_inc(s_dma, 16)
            sync.wait_ge(s_dma, 80)

    bs_cols = np.broadcast_to(
        b64[2:NSPEC + 1].astype(np.float32)[None, :], (ROWS, NSPEC - 1))
    in_maps = []
    for c in range(NCORES):
        yzc_ = Yz[c * ROWS:(c + 1) * ROWS]
        hdr_np = np.concatenate([
            np.full((ROWS, 1), 1.0, np.float32),
            np.full((ROWS, 1), c0, np.float32),
            np.full((ROWS, 1), bt1, np.float32),
            bs_cols,
            yzc_[:, 0:10],
            LNSB[:, 0:10],
            yzc_[:, 10:],
            LNSB[:, 10:],
        ], axis=1)
        in_maps.append({"hdr": np.ascontiguousarray(hdr_np), "wpy": WPYB})
    return nc, in_maps


def kernel(y, W, s, b):
    from concourse.bass_utils import run_bass_kernel_spmd

    nc, in_maps = build(y, W, s, b)
    res = run_bass_kernel_spmd(nc, in_maps, list(range(NCORES))).results
    X = np.concatenate([res[c]["xout"] for c in range(NCORES)], axis=0)
    return X.astype(np.float32)


if __name__ == "__main__":
    rng = np.random.default_rng(0)
    y = rng.standard_normal((B, D)).astype(np.float32)
    W = np.tril(rng.standard_normal((32, 32)), -1).astype(np.float32) * 0.5
    s = rng.standard_normal(D).astype(np.float32)
    b = rng.standard_normal(D).astype(np.float32)
    X = kernel(y=y, W=W, s=s, b=b)
    print("out", X.shape, X.dtype, X[0, :4])
